# revision 1
# baseline (speedup 1.0000x reference)
"""Trainium2 Bass kernel for single-head attention.

Problem: x[8, 2048, 512], Wq/Wk/Wv[512, 512], bq/bk/bv[512] ->
out[8, 2048, 512] where out = softmax((xWq+bq)(xWk+bk)^T / sqrt(512)) (xWv+bv).

Sharding: data-parallel over batch. Each of the 8 NeuronCores computes full
attention for one batch element.

Per-core algorithm (S=2048 sequence, D=512 hidden, P=128 partitions):
  1. Load x [S, D], transpose on the PE (128x128 blocks) to xT [D, S].
  2. QKV projections with the contraction dim (features) on partitions:
       qT/kT [D, S] = W.T @ xT   (bias fused into the PSUM->SBUF evacuation)
       v     [S, D] = x @ Wv     (natural layout; bias via broadcast add)
  3. Scores are computed TRANSPOSED, eT[j, i] = exp(scale * k_j . q_i), so
     that the softmax'd axis j lands on partitions - exactly what the
     attention*V matmul needs as its stationary operand. Softmax skips the
     max-subtraction (scaled scores are ~N(0,1); exp is safe in fp32).
  4. Denominators: eT tiles are accumulated over key-tiles on the DVE
     (esum), one ones-row matmul per query chunk gives den[1, i], the row is
     transposed into per-partition columns with K=1 matmuls, and the output
     tiles are scaled by 1/den during evacuation (split across DVE and ACT).
  5. out[i, h] accumulates sum_j eT[j, i] * v[j, h] over 16 j-tiles in PSUM.

All matmuls run in float32r (TF32-like fast fp32 mode, 4x the fp32 rate,
~1e-4 relative error), accumulating in fp32 PSUM.
"""

import os
import sys

for _p in ("/opt/trn_rl_repo", "/root/.axon_site/_ro/trn_rl_repo"):
    if os.path.isdir(_p) and _p not in sys.path:
        sys.path.append(_p)

import numpy as np

import concourse.bacc as bacc
import concourse.mybir as mybir
import concourse.tile as tile
from concourse.bass_utils import run_bass_kernel_spmd
from concourse.masks import make_identity

B = 8
S = 2048
D = 512
P = 128
NT = S // P  # 16 s-tiles (query/key tiles of 128)
FC = D // P  # 4 feature/hidden chunks of 128
MC = 4  # i-chunks of 512 queries
SCALE = 1.0 / float(np.sqrt(D))

F32 = mybir.dt.float32
F32R = mybir.dt.float32r
ACT_ID = mybir.ActivationFunctionType.Identity
ACT_EXP = mybir.ActivationFunctionType.Exp

_CACHE = {}


def _emit_body(nc, tc, pools, dram, rep, phases="all"):
    """Emit one full attention computation. `rep` uniquifies tile names."""
    consts, w_r_pool, qkT_pool, v_pool, e_pool, out_pool, den_pool = pools
    x, wq, bq, wk, bk, wv, bv, out = dram

    ident = consts["ident"]
    ones32 = consts["ones32"]
    ones_r = consts["ones_r"]
    bq_sb = consts["bq_sb"]
    bk_sb = consts["bk_sb"]
    bv_sb = consts["bv_sb"]
    wq_r = consts["wq_r"]

    # qT_r[c][j] / kT_r[c][j]: hidden-chunk c (128 partitions), query-chunk j
    # (512). Fine-grained tiles so scores can start before all of QKV is done.
    qT_r = [
        [
            qkT_pool.tile([P, 512], F32R, tag=f"qT{c}_{j}", name=f"qT{c}_{j}_{rep}")
            for j in range(FC)
        ]
        for c in range(FC)
    ]
    kT_r = [
        [
            qkT_pool.tile([P, 512], F32R, tag=f"kT{c}_{j}", name=f"kT{c}_{j}_{rep}")
            for j in range(FC)
        ]
        for c in range(FC)
    ]
    v_r = [
        v_pool.tile([P, D], F32R, tag=f"v{t}", name=f"v{t}_{rep}") for t in range(NT)
    ]

    with (
        tc.tile_pool(name="xstage", bufs=2) as xstage,
        tc.tile_pool(name="xT", bufs=2) as xT_pool,
        tc.tile_pool(name="psT", bufs=5, space="PSUM") as psT,
        tc.tile_pool(name="psQKV", bufs=3, space="PSUM") as psQKV,
    ):
        # per s-chunk jj (4 s-tiles = 512 rows): DMA in, transpose, then
        # immediately emit the QKV matmuls that consume just this chunk.
        for jj in range(FC):
            stage = xstage.tile([P, 4, D], F32, tag="stage", name=f"stage{jj}_{rep}")
            nc.sync.dma_start(
                out=stage[:],
                in_=x[jj * 4 * P : (jj + 1) * 4 * P, :].rearrange(
                    "(t p) f -> p t f", p=P
                ),
            )
            if "wk_r" not in consts:
                consts["load_rest"]()
            wk_r = consts["wk_r"]
            wv_r = consts["wv_r"]
            xT_c = [
                xT_pool.tile([P, 512], F32R, tag=f"xT{c}", name=f"xT{c}_{jj}_{rep}")
                for c in range(FC)
            ]
            for tl in range(4):
                for c in range(FC):
                    pst = psT.tile([P, P], F32, tag="pst", name=f"pst{jj}_{tl}_{c}_{rep}")
                    nc.tensor.transpose(
                        pst[:], stage[:, tl, c * P : (c + 1) * P], ident[:]
                    )
                    nc.any.tensor_copy(xT_c[c][:, tl * P : (tl + 1) * P], pst[:])

            # Q^T, K^T for this s-chunk: [h-tile 128, 512]
            for wr, dst, bias in ((wq_r, qT_r, bq_sb), (wk_r, kT_r, bk_sb)):
                for i in range(FC):  # h-tile
                    ps = psQKV.tile(
                        [P, 512], F32, tag="psqk", name=f"psqk{i}_{jj}_{rep}"
                    )
                    for c in range(FC):  # contraction over features
                        nc.tensor.matmul(
                            ps[:],
                            wr[:, c, i * P : (i + 1) * P],
                            xT_c[c][:],
                            start=(c == 0),
                            stop=(c == FC - 1),
                        )
                    nc.scalar.activation(
                        dst[i][jj][:],
                        ps[:],
                        ACT_ID,
                        bias=bias[:, i : i + 1],
                    )

            # V for these 4 s-tiles: [s-tile 128, h]
            for tl in range(4):
                t = jj * 4 + tl
                ps = psQKV.tile([P, 512], F32, tag="psqk", name=f"psv{t}_{rep}")
                for c in range(FC):
                    nc.tensor.matmul(
                        ps[:],
                        xT_c[c][:, tl * P : (tl + 1) * P],
                        wv_r[:, c, :],
                        start=(c == 0),
                        stop=(c == FC - 1),
                    )
                nc.vector.tensor_add(v_r[t][:], ps[:], bv_sb[:])

    if phases == "qkv":
        # ablation: write q/k/v straight out
        for t in range(4):
            o_sb = out_pool.tile([P, D], F32, tag="osb", name=f"oq{t}_{rep}")
            nc.vector.tensor_copy(o_sb[:], v_r[t][:])
            nc.sync.dma_start(out=out[t * P : (t + 1) * P, :], in_=o_sb[:])
        return

    # ---- scores^T -> exp -> denominators + attention * V ----
    with (
        tc.tile_pool(name="psS", bufs=3, space="PSUM") as psS,
        tc.tile_pool(name="psO", bufs=1, space="PSUM") as psO,
        tc.tile_pool(name="psDen", bufs=1, space="PSUM") as psDen,
    ):
        psDenT = psDen
        for m in range(MC):  # chunk of 512 queries
            ps_o = [
                psO.tile([P, D], F32, tag=f"o{t}", name=f"ps_o{t}_{m}_{rep}")
                for t in range(4)
            ]
            ps_den = psDen.tile([1, 512], F32, tag="ps_den", name=f"ps_den{m}_{rep}")
            esum = den_pool.tile([P, 512], F32R, tag="esum", name=f"esum{m}_{rep}")
            for c in range(NT):  # key tile of 128
                ps_s = psS.tile([P, 512], F32, tag="ps_s", name=f"ps_s{m}_{c}_{rep}")
                for hc in range(FC):  # contraction over hidden
                    nc.tensor.matmul(
                        ps_s[:],
                        kT_r[hc][c // 4][:, (c % 4) * P : (c % 4 + 1) * P],
                        qT_r[hc][m][:],
                        start=(hc == 0),
                        stop=(hc == FC - 1),
                    )
                eT = e_pool.tile([P, 512], F32R, tag="eT", name=f"eT{m}_{c}_{rep}")
                nc.scalar.activation(eT[:], ps_s[:], ACT_EXP, scale=SCALE)
                # accumulate eT over key tiles on the (otherwise idle) DVE;
                # one ones-matmul per m-chunk then yields the denominators.
                if c == 0:
                    nc.vector.tensor_copy(esum[:], eT[:])
                else:
                    nc.vector.tensor_add(esum[:], esum[:], eT[:])
                # out[i, h] += eT[j, i-tile].T @ v[j, h]
                for t in range(4):
                    nc.tensor.matmul(
                        ps_o[t][:],
                        eT[:, t * P : (t + 1) * P],
                        v_r[c][:],
                        start=(c == 0),
                        stop=(c == NT - 1),
                    )

            # denominator row: den[1, i] = sum_j esum[j, i]
            nc.tensor.matmul(ps_den[:], ones_r[:], esum[:], start=True, stop=True)
            # transpose the denominator row into per-partition columns with
            # K=1 matmuls, then reciprocal.
            den_row = den_pool.tile([1, 512], F32, tag="den_row", name=f"dr{m}_{rep}")
            nc.vector.tensor_copy(den_row[:], ps_den[:])
            ps_denT = psDenT.tile([P, 4], F32, tag="ps_den", name=f"ps_denT{m}_{rep}")
            for t in range(4):
                nc.tensor.matmul(
                    ps_denT[:, t : t + 1],
                    den_row[:, t * P : (t + 1) * P],
                    ones32[:1, :],
                    start=True,
                    stop=True,
                )
            rec = den_pool.tile([P, 4], F32, tag="rec", name=f"rec{m}_{rep}")
            nc.vector.reciprocal(rec[:], ps_denT[:])

            for t in range(4):
                o_sb = out_pool.tile([P, D], F32, tag="osb", name=f"o{m}_{t}_{rep}")
                if t < 2:
                    nc.vector.tensor_scalar_mul(
                        o_sb[:], ps_o[t][:], rec[:, t : t + 1]
                    )
                else:
                    nc.scalar.activation(
                        o_sb[:], ps_o[t][:], ACT_ID, scale=rec[:, t : t + 1]
                    )
                it = m * 4 + t
                nc.sync.dma_start(out=out[it * P : (it + 1) * P, :], in_=o_sb[:])


def _build_nc(reps=1, phases="all"):
    nc = bacc.Bacc(None)

    x = nc.dram_tensor("x", [S, D], F32, kind="ExternalInput")
    wq = nc.dram_tensor("Wq", [D, D], F32, kind="ExternalInput")
    bq = nc.dram_tensor("bq", [D], F32, kind="ExternalInput")
    wk = nc.dram_tensor("Wk", [D, D], F32, kind="ExternalInput")
    bk = nc.dram_tensor("bk", [D], F32, kind="ExternalInput")
    wv = nc.dram_tensor("Wv", [D, D], F32, kind="ExternalInput")
    bv = nc.dram_tensor("bv", [D], F32, kind="ExternalInput")
    out = nc.dram_tensor("out", [S, D], F32, kind="ExternalOutput")
    dram = (x, wq, bq, wk, bk, wv, bv, out)

    with tile.TileContext(nc) as tc:
        with (
            tc.tile_pool(name="consts", bufs=1) as consts_pool,
            tc.tile_pool(name="w_r", bufs=1) as w_r_pool,
            tc.tile_pool(name="qkT", bufs=1) as qkT_pool,
            tc.tile_pool(name="v", bufs=1) as v_pool,
            tc.tile_pool(name="e", bufs=4) as e_pool,
            tc.tile_pool(name="outsb", bufs=4) as out_pool,
            tc.tile_pool(name="den", bufs=2) as den_pool,
        ):
            consts = {}
            ident = consts_pool.tile([P, P], F32, tag="ident", name="ident")
            make_identity(nc, ident[:])
            consts["ident"] = ident

            ones32 = consts_pool.tile([P, 1], F32, tag="ones32", name="ones32")
            nc.vector.memset(ones32[:], 1.0)
            ones_r = consts_pool.tile([P, 1], F32R, tag="ones_r", name="ones_r")
            nc.vector.tensor_copy(ones_r[:], ones32[:])
            consts["ones32"] = ones32
            consts["ones_r"] = ones_r

            bq_sb = consts_pool.tile([P, FC], F32, tag="bq", name="bq_sb")
            bk_sb = consts_pool.tile([P, FC], F32, tag="bk", name="bk_sb")
            nc.gpsimd.dma_start(out=bq_sb[:], in_=bq.rearrange("(c p) -> p c", p=P))
            nc.gpsimd.dma_start(out=bk_sb[:], in_=bk.rearrange("(c p) -> p c", p=P))
            bv_sb = consts_pool.tile([P, D], F32, tag="bv", name="bv_sb")
            nc.gpsimd.dma_start(out=bv_sb[:], in_=bv[:].partition_broadcast(P))
            consts["bq_sb"] = bq_sb
            consts["bk_sb"] = bk_sb
            consts["bv_sb"] = bv_sb

            wstage_cm = tc.tile_pool(name="wstage", bufs=1)
            wstage = wstage_cm.__enter__()

            def _load_w(wi, wname, wdram):
                stage = wstage.tile(
                    [P, FC, D], F32, tag="wstage", name=f"wstage{wi}"
                )
                nc.sync.dma_start(
                    out=stage[:], in_=wdram.rearrange("(c p) h -> p c h", p=P)
                )
                wr = w_r_pool.tile([P, FC, D], F32R, tag=f"w{wi}", name=f"w{wi}")
                nc.vector.tensor_copy(wr[:], stage[:])
                consts[wname] = wr

            _load_w(0, "wq_r", wq)

            def _load_rest():
                _load_w(1, "wk_r", wk)
                _load_w(2, "wv_r", wv)

            consts["load_rest"] = _load_rest

            pools = (
                consts, w_r_pool, qkT_pool, v_pool, e_pool, out_pool, den_pool,
            )
            for rep in range(reps):
                _emit_body(nc, tc, pools, dram, rep, phases=phases)
            wstage_cm.__exit__(None, None, None)

    nc.finalize()
    return nc


def kernel(x, Wq, bq, Wk, bk, Wv, bv):
    x = np.ascontiguousarray(np.asarray(x, dtype=np.float32))
    args = {
        "Wq": np.ascontiguousarray(np.asarray(Wq, dtype=np.float32)),
        "bq": np.ascontiguousarray(np.asarray(bq, dtype=np.float32)),
        "Wk": np.ascontiguousarray(np.asarray(Wk, dtype=np.float32)),
        "bk": np.ascontiguousarray(np.asarray(bk, dtype=np.float32)),
        "Wv": np.ascontiguousarray(np.asarray(Wv, dtype=np.float32)),
        "bv": np.ascontiguousarray(np.asarray(bv, dtype=np.float32)),
    }

    if "nc" not in _CACHE:
        _CACHE["nc"] = _build_nc()
    nc = _CACHE["nc"]

    in_maps = [{"x": x[b], **args} for b in range(B)]
    try:
        res = run_bass_kernel_spmd(nc, in_maps, list(range(B)))
    except Exception:
        # transient device wedge (e.g. NRT_EXEC_UNIT_UNRECOVERABLE) - retry
        import time as _time

        _time.sleep(5)
        res = run_bass_kernel_spmd(nc, in_maps, list(range(B)))
    return np.stack([res.results[b]["out"] for b in range(B)]).astype(np.float32)


if __name__ == "__main__":
    rng = np.random.default_rng(0)
    inputs = {
        "x": rng.standard_normal((B, S, D), dtype=np.float32),
        "Wq": rng.standard_normal((D, D), dtype=np.float32) / np.sqrt(D),
        "bq": rng.standard_normal(D).astype(np.float32) * 0.01,
        "Wk": rng.standard_normal((D, D), dtype=np.float32) / np.sqrt(D),
        "bk": rng.standard_normal(D).astype(np.float32) * 0.01,
        "Wv": rng.standard_normal((D, D), dtype=np.float32) / np.sqrt(D),
        "bv": rng.standard_normal(D).astype(np.float32) * 0.01,
    }
    got = kernel(**inputs)
    print("kernel output", got.shape, got.dtype)



# revision 51
# speedup vs baseline: 1.3539x; 1.3539x over previous
"""Trainium2 Bass kernel for single-head attention (fp8 DoubleRow edition).

Problem: x[8, 2048, 512], Wq/Wk/Wv[512, 512], bq/bk/bv[512] ->
out[8, 2048, 512] where out = softmax((xWq+bq)(xWk+bk)^T / sqrt(512)) (xWv+bv).

Sharding: data-parallel over batch; each of the 8 NeuronCores does one batch
element.

Algebraic restructure (host folds weights only):
  scores_ij = (x_i Wq + bq)(x_j Wk + bk)^T
            = x_i A x_j^T + (x_i Wq bk) + x_j (Wk bq) + bq.bk     A = Wq Wk^T
  The 2nd and 4th terms are constant per query row i -> softmax-invariant ->
  dropped. The device computes t = x @ A (ONE projection instead of Q and K),
  a per-key logit bias c = x @ (Wk bq) folded into the exp's bias operand,
  and v = x @ Wv. The output bias bv is also softmax-affine:
  out = num/den + bv = (num + den x bv)/den, so den x bv is added to the
  attention*V PSUM with a rank-1 (K=1) matmul and bv never touches the
  element-wise engines.

All big matmuls run as fp8e4m3 DoubleRow (perf_mode=DoubleRow: K=256 per
instruction at 0.5 cycles per output column = 4x the f32r MAC rate). fp8
alone is far too coarse (max-rel ~5e-2 vs the 2e-2 gate), so every operand
is a value+residual pair, residual8 = fp8(value - value8) (the PE's fp8
subnormal arithmetic is exact, so residuals need no rescaling). Each
pair x pair product drops the (residual x residual) term -> 3 passes
instead of 4, i.e. 0.75 cycles per K128-row vs 1.0 for f32r/bf16, at ~0.1%
effective precision:
  t   = x8@A8 + dx8@A8 + x8@dA8        (A, Wv prescaled by 32 so their
  v   = x8@Wv8 + dx8@Wv8 + x8@dWv8      residuals stay in fp8 range;
  sT  = x8.t8 + x8.dt8 + dx8.t8         evacuation rescales by 1/32)
  num = (e8+de8)^T v8 + e8^T dv8
  den = sum_j (e8+de8)  via fp8-ones DoubleRow matmuls into a [1,512] row
e8 = fp8(e16), de8 = fp8(e16-e8), e16 = exp(SCALE*sT + c - SHIFT) in fp16.
SHIFT keeps exp below 240 (fp8e4m3 max finite) and cancels in num/den.

Engine budget per score tile: exp on ACT (no fast modes), fp16->fp8 copy on
Pool, mixed-dtype residual subtract on DVE; the phase-2 emission order runs
scores of tile-pair jp+1 ahead of attention*V of jp so the PE never waits
on that chain (PE stalls would also reset its pstate ramp). Host-side input
prep: x[b]^T is split into the fp8 pair (x8, dx8) and laid out exactly as
the SBUF tiles so every DMA line is one 4KB contiguous descriptor per
partition.
"""

import os
import sys

for _p in ("/opt/trn_rl_repo", "/root/.axon_site/_ro/trn_rl_repo"):
    if os.path.isdir(_p) and _p not in sys.path:
        sys.path.append(_p)

import ml_dtypes
import numpy as np

import concourse.bacc as bacc
import concourse.mybir as mybir
import concourse.tile as tile
from concourse.bass_utils import run_bass_kernel_spmd

B = 8
S = 2048
D = 512
P = 128
NT = S // P  # 16 s-tiles
MC = 4  # query chunks of 512
SCALE = 1.0 / float(np.sqrt(D))
SHIFT = 3.0
WS = 32.0  # A/Wv prescale so the weight residuals stay in fp8 normal range
CS = 256.0  # w3 (tiny per-key bias vector) prescale

F32 = mybir.dt.float32
F32R = mybir.dt.float32r
F16 = mybir.dt.float16
F8 = mybir.dt.float8e4
ACT_ID = mybir.ActivationFunctionType.Identity
ACT_EXP = mybir.ActivationFunctionType.Exp
DR = mybir.MatmulPerfMode.DoubleRow
FP8NP = ml_dtypes.float8_e4m3

_CACHE = {}


def _build_nc():
    nc = bacc.Bacc(None)

    # DRAM layouts mirror the SBUF tiles: [ff, p, u, cols] so each partition
    # line is one contiguous run.
    x8d = nc.dram_tensor("x8", [2, P, 2, S], F8, kind="ExternalInput")
    dx8d = nc.dram_tensor("dx8", [2, P, 2, S], F8, kind="ExternalInput")
    A8d = nc.dram_tensor("A8", [2, P, 2, D], F8, kind="ExternalInput")
    dA8d = nc.dram_tensor("dA8", [2, P, 2, D], F8, kind="ExternalInput")
    Wv8d = nc.dram_tensor("Wv8", [2, P, 2, D], F8, kind="ExternalInput")
    dWv8d = nc.dram_tensor("dWv8", [2, P, 2, D], F8, kind="ExternalInput")
    w38d = nc.dram_tensor("w38", [2, P, 2, 1], F8, kind="ExternalInput")
    bvd = nc.dram_tensor("bv16", [1, D], F16, kind="ExternalInput")
    out = nc.dram_tensor("out", [S, D], F16, kind="ExternalOutput")

    with tile.TileContext(nc) as tc:
        lp = nc.allow_low_precision(
            reason="fp8 value+residual pairs carry ~0.1% effective precision"
        )
        lp.__enter__()
        with (
            tc.tile_pool(name="consts", bufs=1) as cp,
            tc.tile_pool(name="tv8", bufs=1) as tv8,
            tc.tile_pool(name="e8p", bufs=5) as e8p,
            tc.tile_pool(name="e16p", bufs=4) as e16p,
            tc.tile_pool(name="tv16", bufs=3) as tv16,
            tc.tile_pool(name="denp", bufs=2) as denp,
            tc.tile_pool(name="outp", bufs=4) as outp,
        ):
            # ---- inputs to SBUF ----
            # DMA order matters: the first projection tiles need x8+A8, then
            # Wv8 (v pass 1), then the residual tensors.
            def mk2(name, cols):
                return [
                    cp.tile([P, 2, cols], F8, tag=f"{name}{ff}", name=f"{name}{ff}")
                    for ff in range(2)
                ]

            x8sb = mk2("x8", S)
            A8sb = mk2("A8", D)
            dx8sb = mk2("dx8", S)
            dA8sb = mk2("dA8", D)
            Wv8sb = mk2("Wv8", D)
            dWv8sb = mk2("dWv8", D)
            w38sb = mk2("w38", 1)

            def load(ts_, dram, ffs=(0, 1)):
                for ff in ffs:
                    nc.sync.dma_start(out=ts_[ff][:], in_=dram[ff])

            load(x8sb, x8d, (0,))
            load(A8sb, A8d)
            load(x8sb, x8d, (1,))
            load(Wv8sb, Wv8d)
            load(dx8sb, dx8d)
            load(dA8sb, dA8d)
            load(dWv8sb, dWv8d)
            load(w38sb, w38d)

            # bv broadcast to all partitions as f16 (added at out-evacuation;
            # f16 everywhere makes the add eligible for the DVE 2x mode)
            bv_sb = cp.tile([P, D], F16, tag="bv", name="bv_sb")
            nc.gpsimd.dma_start(out=bv_sb[:], in_=bvd[0, :].partition_broadcast(P))

            # all-ones [P, 2, 128] stationary for the denominator matmuls
            # (every output partition gets the same column sum; single-row
            # stationaries fail walrus' ldweights ISA check)
            ones32 = cp.tile([P, 2, P], F32, tag="ones32", name="ones32")
            nc.vector.memset(ones32[:], 1.0)
            ones8 = cp.tile([P, 2, P], F8, tag="ones8", name="ones8")
            nc.vector.tensor_copy(ones8[:], ones32[:])
            mov_one = cp.tile([1, 1], F32, tag="mov_one", name="mov_one")
            nc.vector.tensor_copy(mov_one[:], ones32[:1, 0, :1])

            t8sb = [
                tv8.tile([P, 2, S], F8, tag=f"t8_{gg}", name=f"t8_{gg}")
                for gg in range(2)
            ]
            dt8sb = [
                tv8.tile([P, 2, S], F8, tag=f"dt8_{gg}", name=f"dt8_{gg}")
                for gg in range(2)
            ]
            v8sb = [
                tv8.tile([P, 2, D], F8, tag=f"v8_{jp}", name=f"v8_{jp}")
                for jp in range(8)
            ]
            dv8sb = [
                tv8.tile([P, 2, D], F8, tag=f"dv8_{jp}", name=f"dv8_{jp}")
                for jp in range(8)
            ]
            cT_sb = cp.tile([P, NT], F32, tag="cT", name="cT_sb")

            # ---- phase 1: projections (t, v), then the tiny c bias ----
            # tiles are processed in PAIRS: one [P, 2, D] two-bank PSUM tile
            # per pair -> one ACT evac + one fp8 copy + one residual subtract.
            with (
                tc.tile_pool(name="psP", bufs=3, space="PSUM") as psP,
                tc.tile_pool(name="psC", bufs=1, space="PSUM") as psC,
            ):
                evac_n = [0]

                def emit_pair_evac(ps2, sl8, dsl8, name):
                    # ps2 ([P,2,D] f32 PSUM, 32x-scaled) -> fp8 pair via f16
                    t16 = tv16.tile([P, 2, D], F16, tag="t16", name=f"t16_{name}")
                    nc.scalar.activation(t16[:], ps2[:], ACT_ID, scale=1.0 / WS)
                    # balance the fp8-copy between Pool (slow) and DVE (2x mode)
                    if evac_n[0] % 4 < 3:
                        nc.gpsimd.tensor_copy(sl8, t16[:])
                    else:
                        nc.vector.tensor_copy(sl8, t16[:])
                    evac_n[0] += 1
                    nc.vector.tensor_sub(dsl8, t16[:], sl8)

                def emit_t(gg, mc):
                    # pair of g-tiles (2gg, 2gg+1) for query-chunk mc
                    ps2 = psP.tile([P, 2, D], F32, tag="psp", name=f"ps_t{gg}_{mc}")
                    msl = slice(mc * D, (mc + 1) * D)
                    for u in range(2):
                        gt = 2 * gg + u
                        gsl = slice(gt * P, (gt + 1) * P)
                        k = 0
                        for stat, mov in (
                            (A8sb, x8sb),
                            (A8sb, dx8sb),
                            (dA8sb, x8sb),
                        ):
                            for ff in range(2):
                                nc.tensor.matmul(
                                    ps2[:, u, :],
                                    stat[ff][:, :, gsl],
                                    mov[ff][:, :, msl],
                                    start=(k == 0),
                                    stop=(k == 5),
                                    perf_mode=DR,
                                )
                                k += 1
                    emit_pair_evac(
                        ps2,
                        t8sb[gg][:, :, msl],
                        dt8sb[gg][:, :, msl],
                        f"t{gg}_{mc}",
                    )

                def emit_v(jp):
                    # pair of s-tiles (2jp, 2jp+1): fills v8sb[jp]/dv8sb[jp]
                    ps2 = psP.tile([P, 2, D], F32, tag="psp", name=f"ps_v{jp}")
                    for u in range(2):
                        st = 2 * jp + u
                        ssl = slice(st * P, (st + 1) * P)
                        k = 0
                        for stat, mov in (
                            (x8sb, Wv8sb),
                            (dx8sb, Wv8sb),
                            (x8sb, dWv8sb),
                        ):
                            for ff in range(2):
                                nc.tensor.matmul(
                                    ps2[:, u, :],
                                    stat[ff][:, :, ssl],
                                    mov[ff][:],
                                    start=(k == 0),
                                    stop=(k == 5),
                                    perf_mode=DR,
                                )
                                k += 1

                    emit_pair_evac(
                        ps2, v8sb[jp][:], dv8sb[jp][:], f"v{jp}"
                    )

                def emit_c():
                    # per-key logit bias cT[j, tile] = x8 . w38 (256c), landed
                    # directly with keys on partitions via x8-stationary plain
                    # fp8 matmuls (K=128 each, N=1)
                    ps_ct = psC.tile([P, NT], F32, tag="ps_ct", name="ps_ct")
                    for t in range(NT):
                        ssl = slice(t * P, (t + 1) * P)
                        k = 0
                        for ff in range(2):
                            for u in range(2):
                                nc.tensor.matmul(
                                    ps_ct[:, t : t + 1],
                                    x8sb[ff][:, u, ssl],
                                    w38sb[ff][:, u, :],
                                    start=(k == 0),
                                    stop=(k == 3),
                                )
                                k += 1
                    nc.vector.tensor_scalar(
                        cT_sb[:],
                        ps_ct[:],
                        SCALE / CS,
                        -SHIFT,
                        mybir.AluOpType.mult,
                        mybir.AluOpType.add,
                    )

                # interleave so the PE always has a DMA-satisfied group ready
                order = [("t", 0, 0), ("v", 0, None),
                         ("t", 1, 0), ("v", 1, None),
                         ("c", None, None), ("v", 2, None), ("v", 3, None)]
                order += [("v", jp, None) for jp in range(4, 8)]
                order += [("t", gg, mc) for mc in range(1, MC) for gg in range(2)]
                for kind_, a, b in order:
                    if kind_ == "t":
                        emit_t(a, b)
                    elif kind_ == "c":
                        emit_c()
                    else:
                        emit_v(a)

            # ---- phase 2: scores -> exp pair -> AV + den, software-pipelined ----
            with (
                tc.tile_pool(name="psS", bufs=2, space="PSUM") as psS,
                tc.tile_pool(name="psO", bufs=1, space="PSUM") as psO,
                tc.tile_pool(name="psDen", bufs=1, space="PSUM") as psDen,
                tc.tile_pool(name="psR", bufs=1, space="PSUM") as psR,
            ):
                state = {}  # per-m live objects

                def emit_scores(m, jp):
                    st = state[m]
                    e8t = e8p.tile([P, 2, D], F8, tag="e8", name=f"e8_{m}_{jp}")
                    de8t = e8p.tile([P, 2, D], F8, tag="de8", name=f"de8_{m}_{jp}")
                    e16 = e16p.tile([P, 2, D], F16, tag="e16", name=f"e16_{m}_{jp}")
                    for u in range(2):
                        j16 = jp * 2 + u
                        jsl = slice(j16 * P, (j16 + 1) * P)
                        msl = slice(m * D, (m + 1) * D)
                        ps_s = psS.tile([P, D], F32, tag="s", name=f"ps_s{m}_{j16}")
                        k = 0
                        for stat, mov in (
                            (x8sb, t8sb),
                            (x8sb, dt8sb),
                            (dx8sb, t8sb),
                        ):
                            for ff in range(2):
                                nc.tensor.matmul(
                                    ps_s[:],
                                    stat[ff][:, :, jsl],
                                    mov[ff][:, :, msl],
                                    start=(k == 0),
                                    stop=(k == 5),
                                    perf_mode=DR,
                                )
                                k += 1
                        nc.scalar.activation(
                            e16[:, u, :],
                            ps_s[:],
                            ACT_EXP,
                            scale=SCALE,
                            bias=cT_sb[:, j16 : j16 + 1],
                        )
                    nc.vector.tensor_copy(e8t[:], e16[:])
                    nc.vector.tensor_sub(de8t[:], e16[:], e8t[:])
                    st.setdefault("pend", []).append((jp, e8t, de8t))

                def emit_av(m):
                    st = state[m]
                    jp, e8t, de8t = st["pend"].pop(0)
                    for tq in range(4):
                        sl = slice(tq * P, (tq + 1) * P)
                        for pi, (stat, mov) in enumerate((
                            (e8t, v8sb[jp]),
                            (de8t, v8sb[jp]),
                            (e8t, dv8sb[jp]),
                        )):
                            nc.tensor.matmul(
                                st["ps_o"][tq][:],
                                stat[:, :, sl],
                                mov[:],
                                start=(jp == 0 and pi == 0),
                                stop=(jp == 7 and pi == 2),
                                perf_mode=DR,
                            )
                    nc.tensor.matmul(
                        st["ps_den"][:],
                        ones8[:],
                        e8t[:],
                        start=(jp == 0),
                        stop=False,
                        perf_mode=DR,
                    )
                    nc.tensor.matmul(
                        st["ps_den"][:],
                        ones8[:],
                        de8t[:],
                        start=False,
                        stop=(jp == 7),
                        perf_mode=DR,
                    )
                    if jp == 7:
                        rec_row = denp.tile([1, D], F32, tag="rr", name=f"rr{m}")
                        nc.vector.reciprocal(rec_row[:], st["ps_den"][0:1, :])
                        st["rec_row"] = rec_row

                def emit_m_tail(m):
                    # transpose 1/den into per-partition columns with a tiny
                    # SBUF->SBUF DMA gather, then scale + DMA out
                    st = state.pop(m)
                    ps_r = psR.tile([P, 4], F32, tag="pr", name=f"ps_r{m}")
                    for tq in range(4):
                        nc.tensor.matmul(
                            ps_r[:, tq : tq + 1],
                            st["rec_row"][:, tq * P : (tq + 1) * P],
                            mov_one[:1, :],
                            start=True,
                            stop=True,
                        )
                    rec_sb = denp.tile([P, 4], F32, tag="rs", name=f"rs{m}")
                    nc.vector.tensor_copy(rec_sb[:], ps_r[:])
                    for tq in range(4):
                        o_sb = outp.tile([P, D], F16, tag="osb", name=f"o{m}_{tq}")
                        rc = rec_sb[:, tq : tq + 1]
                        # spread the four scale ops so the PSUM banks release
                        # in parallel (each frees its bank for the next m).
                        # NB GPSIMD/Pool cannot read PSUM on hardware.
                        if tq in (1, 2):
                            nc.scalar.activation(
                                o_sb[:], st["ps_o"][tq][:], ACT_ID, scale=rc
                            )
                        else:
                            nc.vector.tensor_scalar_mul(o_sb[:], st["ps_o"][tq][:], rc)
                        nc.vector.tensor_add(o_sb[:], o_sb[:], bv_sb[:])
                        it = m * 4 + tq
                        nc.sync.dma_start(
                            out=out[it * P : (it + 1) * P, :], in_=o_sb[:]
                        )

                def open_m(m):
                    state[m] = {
                        "ps_o": [
                            psO.tile([P, D], F32, tag=f"o{t}", name=f"ps_o{t}_{m}")
                            for t in range(4)
                        ],
                        "ps_den": psDen.tile([P, D], F32, tag="dn", name=f"ps_den{m}"),
                    }

                # pipeline: scores(m, jp) run LAG steps ahead of AV(m, jp);
                # the m-tail (which waits on the reciprocal chain) is delayed
                # one further step so the PE never idles on den -> 1/den.
                LAG = 4
                seq = [(m_, jp_) for m_ in range(MC) for jp_ in range(8)]
                open_m(0)
                for k in range(LAG):
                    emit_scores(*seq[k])
                pending_tail = None
                for i, (m, jp) in enumerate(seq):
                    if i + LAG < len(seq):
                        sm, sj = seq[i + LAG]
                        if sj == 0:
                            open_m(sm)
                        emit_scores(sm, sj)
                    if pending_tail is not None:
                        emit_m_tail(pending_tail)
                        pending_tail = None
                    emit_av(m)
                    if jp == 7:
                        pending_tail = m
                emit_m_tail(MC - 1)

        lp.__exit__(None, None, None)

    nc.finalize()
    return nc


def _q8(a):
    return np.ascontiguousarray(a).astype(FP8NP)


def _tile4(a, cols):
    """[512, cols] -> [2, 128, 2, cols] matching the SBUF ff-pair layout."""
    return np.ascontiguousarray(
        a.reshape(2, 2, P, cols).transpose(0, 2, 1, 3)
    )


def _prep_weights(Wq, bq, Wk, bk, Wv, bv):
    Wq = np.asarray(Wq, dtype=np.float64)
    Wk = np.asarray(Wk, dtype=np.float64)
    A = (Wq @ Wk.T).astype(np.float32)
    w3 = (Wk @ np.asarray(bq, dtype=np.float64)).astype(np.float32)
    A8 = _q8(WS * A)
    dA8 = _q8(WS * A - A8.astype(np.float32))
    Wv32 = np.asarray(Wv, dtype=np.float32)
    Wv8 = _q8(WS * Wv32)
    dWv8 = _q8(WS * Wv32 - Wv8.astype(np.float32))
    w38 = _q8(CS * w3)
    return {
        "A8": _tile4(A8, D),
        "dA8": _tile4(dA8, D),
        "Wv8": _tile4(Wv8, D),
        "dWv8": _tile4(dWv8, D),
        "w38": _tile4(w38, 1),
        "bv16": np.ascontiguousarray(
            np.asarray(bv, dtype=np.float16).reshape(1, D)
        ),
    }


def kernel(x, Wq, bq, Wk, bk, Wv, bv):
    x = np.asarray(x, dtype=np.float32)
    wargs = _prep_weights(Wq, bq, Wk, bk, Wv, bv)

    if "nc" not in _CACHE:
        _CACHE["nc"] = _build_nc()
    nc = _CACHE["nc"]

    in_maps = []
    for b in range(B):
        xT = np.ascontiguousarray(x[b].T)
        x8 = xT.astype(FP8NP)
        dx8 = (xT - x8.astype(np.float32)).astype(FP8NP)
        in_maps.append(
            {"x8": _tile4(x8, S), "dx8": _tile4(dx8, S), **wargs}
        )

    try:
        res = run_bass_kernel_spmd(nc, in_maps, list(range(B)))
    except Exception:
        # transient device wedge (e.g. NRT_EXEC_UNIT_UNRECOVERABLE) - retry
        import time as _time

        _time.sleep(5)
        res = run_bass_kernel_spmd(nc, in_maps, list(range(B)))
    return np.stack(
        [np.asarray(res.results[b]["out"]) for b in range(B)]
    ).astype(np.float32)


if __name__ == "__main__":
    rng = np.random.default_rng(0)
    inputs = {
        "x": rng.standard_normal((B, S, D), dtype=np.float32),
        "Wq": rng.standard_normal((D, D), dtype=np.float32) / np.sqrt(D),
        "bq": rng.standard_normal(D).astype(np.float32) * 0.01,
        "Wk": rng.standard_normal((D, D), dtype=np.float32) / np.sqrt(D),
        "bk": rng.standard_normal(D).astype(np.float32) * 0.01,
        "Wv": rng.standard_normal((D, D), dtype=np.float32) / np.sqrt(D),
        "bv": rng.standard_normal(D).astype(np.float32) * 0.01,
    }
    got = kernel(**inputs)
    print("kernel output", got.shape, got.dtype)


# revision 77
# speedup vs baseline: 1.4172x; 1.0468x over previous
"""Trainium2 Bass kernel for single-head attention (fp8 DoubleRow edition).

Problem: x[8, 2048, 512], Wq/Wk/Wv[512, 512], bq/bk/bv[512] ->
out[8, 2048, 512] where out = softmax((xWq+bq)(xWk+bk)^T / sqrt(512)) (xWv+bv).

Sharding: data-parallel over batch; each of the 8 NeuronCores does one batch
element.

Algebraic restructure (host folds weights only):
  scores_ij = (x_i Wq + bq)(x_j Wk + bk)^T
            = x_i A x_j^T + (x_i Wq bk) + x_j (Wk bq) + bq.bk     A = Wq Wk^T
  The 2nd and 4th terms are constant per query row i -> softmax-invariant ->
  dropped. The device computes t = x @ A (ONE projection instead of Q and K),
  a per-key logit bias c = x @ (Wk bq) folded into the exp's bias operand,
  and v = x @ Wv. The output bias bv is softmax-affine (sum of weights = 1)
  and is added as an fp16 broadcast row at output evacuation.

All big matmuls run as fp8e4m3 DoubleRow (perf_mode=DoubleRow: K=256 per
instruction at 0.5 cycles per output column = 4x the f32r MAC rate). fp8
alone is far too coarse (max-rel ~5e-2 vs the 2e-2 gate), so every operand
is a value+residual pair, residual8 = fp8(value - value8) (the PE's fp8
subnormal arithmetic is exact, so residuals need no rescaling). Each
pair x pair product drops the (residual x residual) term -> 3 passes
instead of 4, i.e. 0.75 cycles per K128-row vs 1.0 for f32r/bf16, at ~0.1%
effective precision:
  t   = x8@A8 + dx8@A8 + x8@dA8        (A, Wv prescaled by 32 so their
  v   = x8@Wv8 + dx8@Wv8 + x8@dWv8      residuals stay in fp8 range;
  sT  = x8.t8 + x8.dt8 + dx8.t8         evacuation rescales by 1/32)
  num = (e8+de8)^T v8 + e8^T dv8
  den = sum_j (e8+de8)  via fp8-ones DoubleRow matmuls into a [1,512] row
e8 = fp8(e16), de8 = fp8(e16-e8), e16 = exp(SCALE*sT + c - SHIFT) in fp16.
SHIFT keeps exp below 240 (fp8e4m3 max finite) and cancels in num/den.

Engine budget: exp on ACT (no fast modes); the fp16 intermediates make the
fp8 copies / residual subtracts eligible for the DVE 2x modes, with Pool
taking a share of the phase-1 copies. Tiles are evacuated in PAIRS (one
[P,2,D] two-bank PSUM tile -> one ACT evac + one copy + one subtract).
The phase-2 emission order runs scores LAG=5 tile-pairs ahead of
attention*V and delays each m-chunk's den->1/den->evacuate tail by one
step, so the PE never waits on those chains (PE stalls would also reset
its pstate ramp to 1.2GHz). Host-side input prep: x[b]^T is split into the
fp8 pair (x8, dx8) and laid out exactly as the SBUF tiles so every DMA
line is one 4KB contiguous run per partition.

Hardware-verified: rel err 3.24e-3 vs the fp32 reference; walrus/ISA
constraints found on the way: GPSIMD cannot touch PSUM, DR matmuls need
>=128-partition stationaries (single-row ldweights fails the ISA check),
f32r K=1 transpose matmuls and fp8/f32r memsets are rejected (use f32 +
tensor_copy), and SBUF->SBUF partition-scatter DMA gathers return garbage
on hardware (use K=1 f32 transpose matmuls instead).
"""

import os
import sys

for _p in ("/opt/trn_rl_repo", "/root/.axon_site/_ro/trn_rl_repo"):
    if os.path.isdir(_p) and _p not in sys.path:
        sys.path.append(_p)

import ml_dtypes
import numpy as np

import concourse.bacc as bacc
import concourse.mybir as mybir
import concourse.tile as tile
from concourse.bass_utils import run_bass_kernel_spmd

B = 8
S = 2048
D = 512
P = 128
NT = S // P  # 16 s-tiles
MC = 4  # query chunks of 512
SCALE = 1.0 / float(np.sqrt(D))
SHIFT = 3.0
WS = 32.0  # A/Wv prescale so the weight residuals stay in fp8 normal range
CS = 256.0  # w3 (tiny per-key bias vector) prescale

F32 = mybir.dt.float32
F32R = mybir.dt.float32r
F16 = mybir.dt.float16
F8 = mybir.dt.float8e4
ACT_ID = mybir.ActivationFunctionType.Identity
ACT_EXP = mybir.ActivationFunctionType.Exp
DR = mybir.MatmulPerfMode.DoubleRow
FP8NP = ml_dtypes.float8_e4m3

_CACHE = {}


def _build_nc():
    nc = bacc.Bacc(None)

    # DRAM layouts mirror the SBUF tiles: [ff, p, u, cols] so each partition
    # line is one contiguous run.
    x8d = nc.dram_tensor("x8", [2, P, 2, S], F8, kind="ExternalInput")
    dx8d = nc.dram_tensor("dx8", [2, P, 2, S], F8, kind="ExternalInput")
    A8d = nc.dram_tensor("A8", [2, P, 2, D], F8, kind="ExternalInput")
    dA8d = nc.dram_tensor("dA8", [2, P, 2, D], F8, kind="ExternalInput")
    Wv8d = nc.dram_tensor("Wv8", [2, P, 2, D], F8, kind="ExternalInput")
    dWv8d = nc.dram_tensor("dWv8", [2, P, 2, D], F8, kind="ExternalInput")
    w38d = nc.dram_tensor("w38", [2, P, 2, 1], F8, kind="ExternalInput")
    bvd = nc.dram_tensor("bv16", [1, D], F16, kind="ExternalInput")
    out = nc.dram_tensor("out", [S, D], F16, kind="ExternalOutput")

    with tile.TileContext(nc) as tc:
        lp = nc.allow_low_precision(
            reason="fp8 value+residual pairs carry ~0.1% effective precision"
        )
        lp.__enter__()
        with (
            tc.tile_pool(name="consts", bufs=1) as cp,
            tc.tile_pool(name="tv8", bufs=1) as tv8,
            tc.tile_pool(name="e8p", bufs=5) as e8p,
            tc.tile_pool(name="e16p", bufs=4) as e16p,
            tc.tile_pool(name="tv16", bufs=6) as tv16,
            tc.tile_pool(name="denp", bufs=2) as denp,
            tc.tile_pool(name="outp", bufs=4) as outp,
        ):
            # ---- inputs to SBUF ----
            # DMA order matters: the first projection tiles need x8+A8, then
            # Wv8 (v pass 1), then the residual tensors.
            def mk2(name, cols):
                return [
                    cp.tile([P, 2, cols], F8, tag=f"{name}{ff}", name=f"{name}{ff}")
                    for ff in range(2)
                ]

            x8sb = mk2("x8", S)
            A8sb = mk2("A8", D)
            dx8sb = mk2("dx8", S)
            dA8sb = mk2("dA8", D)
            Wv8sb = mk2("Wv8", D)
            dWv8sb = mk2("dWv8", D)
            w38sb = mk2("w38", 1)

            def load(ts_, dram, ffs=(0, 1)):
                for ff in ffs:
                    nc.sync.dma_start(out=ts_[ff][:], in_=dram[ff])

            load(x8sb, x8d, (0,))
            load(A8sb, A8d)
            load(x8sb, x8d, (1,))
            load(Wv8sb, Wv8d)
            load(dx8sb, dx8d)
            load(dA8sb, dA8d)
            load(dWv8sb, dWv8d)
            load(w38sb, w38d)

            # bv broadcast to all partitions as f16 (added at out-evacuation;
            # f16 everywhere makes the add eligible for the DVE 2x mode)
            bv_sb = cp.tile([P, D], F16, tag="bv", name="bv_sb")
            nc.gpsimd.dma_start(out=bv_sb[:], in_=bvd[0, :].partition_broadcast(P))

            # all-ones [P, 2, 128] stationary for the denominator matmuls
            # (every output partition gets the same column sum; single-row
            # stationaries fail walrus' ldweights ISA check)
            ones32 = cp.tile([P, 2, P], F32, tag="ones32", name="ones32")
            nc.vector.memset(ones32[:], 1.0)
            ones8 = cp.tile([P, 2, P], F8, tag="ones8", name="ones8")
            nc.vector.tensor_copy(ones8[:], ones32[:])
            mov_one = cp.tile([1, 1], F32, tag="mov_one", name="mov_one")
            nc.vector.tensor_copy(mov_one[:], ones32[:1, 0, :1])

            t8sb = [
                tv8.tile([P, 2, S], F8, tag=f"t8_{gg}", name=f"t8_{gg}")
                for gg in range(2)
            ]
            dt8sb = [
                tv8.tile([P, 2, S], F8, tag=f"dt8_{gg}", name=f"dt8_{gg}")
                for gg in range(2)
            ]
            v8sb = [
                tv8.tile([P, 2, D], F8, tag=f"v8_{jp}", name=f"v8_{jp}")
                for jp in range(8)
            ]
            dv8sb = [
                tv8.tile([P, 2, D], F8, tag=f"dv8_{jp}", name=f"dv8_{jp}")
                for jp in range(8)
            ]
            cT_sb = cp.tile([P, NT], F32, tag="cT", name="cT_sb")

            # ---- phase 1: projections (t, v), then the tiny c bias ----
            # tiles are processed in PAIRS: one [P, 2, D] two-bank PSUM tile
            # per pair -> one ACT evac + one fp8 copy + one residual subtract.
            with (
                tc.tile_pool(name="psP", bufs=3, space="PSUM") as psP,
                tc.tile_pool(name="psC", bufs=1, space="PSUM") as psC,
            ):
                evac_n = [0]

                def emit_pair_evac(ps2, sl8, dsl8, name):
                    # ps2 ([P,2,D] f32 PSUM, 32x-scaled) -> fp8 pair via f16
                    t16 = tv16.tile([P, 2, D], F16, tag="t16", name=f"t16_{name}")
                    nc.scalar.activation(t16[:], ps2[:], ACT_ID, scale=1.0 / WS)
                    # balance the fp8-copy between Pool (slow) and DVE (2x mode)
                    if evac_n[0] % 4 < 3:
                        nc.gpsimd.tensor_copy(sl8, t16[:])
                    else:
                        nc.vector.tensor_copy(sl8, t16[:])
                    evac_n[0] += 1
                    nc.vector.tensor_sub(dsl8, t16[:], sl8)

                def emit_t(gg, mc):
                    # pair of g-tiles (2gg, 2gg+1) for query-chunk mc
                    ps2 = psP.tile([P, 2, D], F32, tag="psp", name=f"ps_t{gg}_{mc}")
                    msl = slice(mc * D, (mc + 1) * D)
                    for u in range(2):
                        gt = 2 * gg + u
                        gsl = slice(gt * P, (gt + 1) * P)
                        k = 0
                        for stat, mov in (
                            (A8sb, x8sb),
                            (A8sb, dx8sb),
                            (dA8sb, x8sb),
                        ):
                            for ff in range(2):
                                nc.tensor.matmul(
                                    ps2[:, u, :],
                                    stat[ff][:, :, gsl],
                                    mov[ff][:, :, msl],
                                    start=(k == 0),
                                    stop=(k == 5),
                                    perf_mode=DR,
                                )
                                k += 1
                    emit_pair_evac(
                        ps2,
                        t8sb[gg][:, :, msl],
                        dt8sb[gg][:, :, msl],
                        f"t{gg}_{mc}",
                    )

                def emit_v(jp):
                    # pair of s-tiles (2jp, 2jp+1): fills v8sb[jp]/dv8sb[jp]
                    ps2 = psP.tile([P, 2, D], F32, tag="psp", name=f"ps_v{jp}")
                    for u in range(2):
                        st = 2 * jp + u
                        ssl = slice(st * P, (st + 1) * P)
                        k = 0
                        for stat, mov in (
                            (x8sb, Wv8sb),
                            (dx8sb, Wv8sb),
                            (x8sb, dWv8sb),
                        ):
                            for ff in range(2):
                                nc.tensor.matmul(
                                    ps2[:, u, :],
                                    stat[ff][:, :, ssl],
                                    mov[ff][:],
                                    start=(k == 0),
                                    stop=(k == 5),
                                    perf_mode=DR,
                                )
                                k += 1

                    emit_pair_evac(
                        ps2, v8sb[jp][:], dv8sb[jp][:], f"v{jp}"
                    )

                def emit_c():
                    # per-key logit bias cT[j, tile] = x8 . w38 (256c), landed
                    # directly with keys on partitions via x8-stationary plain
                    # fp8 matmuls (K=128 each, N=1)
                    ps_ct = psC.tile([P, NT], F32, tag="ps_ct", name="ps_ct")
                    for t in range(NT):
                        ssl = slice(t * P, (t + 1) * P)
                        k = 0
                        for ff in range(2):
                            for u in range(2):
                                nc.tensor.matmul(
                                    ps_ct[:, t : t + 1],
                                    x8sb[ff][:, u, ssl],
                                    w38sb[ff][:, u, :],
                                    start=(k == 0),
                                    stop=(k == 3),
                                )
                                k += 1
                    nc.vector.tensor_scalar(
                        cT_sb[:],
                        ps_ct[:],
                        SCALE / CS,
                        -SHIFT,
                        mybir.AluOpType.mult,
                        mybir.AluOpType.add,
                    )

                # interleave so the PE always has a DMA-satisfied group ready
                order = [("t", 0, 0), ("v", 0, None),
                         ("t", 1, 0), ("v", 1, None),
                         ("c", None, None), ("v", 2, None), ("v", 3, None)]
                order += [("v", jp, None) for jp in range(4, 8)]
                order += [("t", gg, mc) for mc in range(1, MC) for gg in range(2)]
                for kind_, a, b in order:
                    if kind_ == "t":
                        emit_t(a, b)
                    elif kind_ == "c":
                        emit_c()
                    else:
                        emit_v(a)

            # ---- phase 2: scores -> exp pair -> AV + den, software-pipelined ----
            with (
                tc.tile_pool(name="psS", bufs=2, space="PSUM") as psS,
                tc.tile_pool(name="psO", bufs=1, space="PSUM") as psO,
                tc.tile_pool(name="psDen", bufs=1, space="PSUM") as psDen,
                tc.tile_pool(name="psR", bufs=1, space="PSUM") as psR,
            ):
                state = {}  # per-m live objects

                def emit_scores(m, jp):
                    st = state[m]
                    e8t = e8p.tile([P, 2, D], F8, tag="e8", name=f"e8_{m}_{jp}")
                    de8t = e8p.tile([P, 2, D], F8, tag="de8", name=f"de8_{m}_{jp}")
                    e16 = e16p.tile([P, 2, D], F16, tag="e16", name=f"e16_{m}_{jp}")
                    for u in range(2):
                        j16 = jp * 2 + u
                        jsl = slice(j16 * P, (j16 + 1) * P)
                        msl = slice(m * D, (m + 1) * D)
                        ps_s = psS.tile([P, D], F32, tag="s", name=f"ps_s{m}_{j16}")
                        k = 0
                        for stat, mov in (
                            (x8sb, t8sb),
                            (x8sb, dt8sb),
                            (dx8sb, t8sb),
                        ):
                            for ff in range(2):
                                nc.tensor.matmul(
                                    ps_s[:],
                                    stat[ff][:, :, jsl],
                                    mov[ff][:, :, msl],
                                    start=(k == 0),
                                    stop=(k == 5),
                                    perf_mode=DR,
                                )
                                k += 1
                        nc.scalar.activation(
                            e16[:, u, :],
                            ps_s[:],
                            ACT_EXP,
                            scale=SCALE,
                            bias=cT_sb[:, j16 : j16 + 1],
                        )
                    nc.vector.tensor_copy(e8t[:], e16[:])
                    nc.vector.tensor_sub(de8t[:], e16[:], e8t[:])
                    # denominator matmuls ride with the scores (LAG ahead of
                    # AV) so den -> 1/den completes long before the m-tail
                    nc.tensor.matmul(
                        st["ps_den"][:],
                        ones8[:],
                        e8t[:],
                        start=(jp == 0),
                        stop=False,
                        perf_mode=DR,
                    )
                    nc.tensor.matmul(
                        st["ps_den"][:],
                        ones8[:],
                        de8t[:],
                        start=False,
                        stop=(jp == 7),
                        perf_mode=DR,
                    )
                    if jp == 7:
                        rec_row = denp.tile([1, D], F32, tag="rr", name=f"rr{m}")
                        nc.vector.reciprocal(rec_row[:], st["ps_den"][0:1, :])
                        ps_r = psR.tile([P, 4], F32, tag="pr", name=f"ps_r{m}")
                        for tq in range(4):
                            nc.tensor.matmul(
                                ps_r[:, tq : tq + 1],
                                rec_row[:, tq * P : (tq + 1) * P],
                                mov_one[:1, :],
                                start=True,
                                stop=True,
                            )
                        rec_sb = denp.tile([P, 4], F32, tag="rs", name=f"rs{m}")
                        nc.vector.tensor_copy(rec_sb[:], ps_r[:])
                        st["rec_sb"] = rec_sb
                    st.setdefault("pend", []).append((jp, e8t, de8t))

                def emit_av(m):
                    st = state[m]
                    jp, e8t, de8t = st["pend"].pop(0)
                    for tq in range(4):
                        sl = slice(tq * P, (tq + 1) * P)
                        for pi, (stat, mov) in enumerate((
                            (e8t, v8sb[jp]),
                            (de8t, v8sb[jp]),
                            (e8t, dv8sb[jp]),
                        )):
                            nc.tensor.matmul(
                                st["ps_o"][tq][:],
                                stat[:, :, sl],
                                mov[:],
                                start=(jp == 0 and pi == 0),
                                stop=(jp == 7 and pi == 2),
                                perf_mode=DR,
                            )

                def emit_m_tail(m):
                    # transpose 1/den into per-partition columns with a tiny
                    # SBUF->SBUF DMA gather, then scale + DMA out
                    st = state.pop(m)
                    rec_sb = st["rec_sb"]
                    o_sb = outp.tile([P, 4, D], F16, tag="osb", name=f"o{m}")
                    for tq in range(4):
                        osl = o_sb[:, tq, :]
                        rc = rec_sb[:, tq : tq + 1]
                        # spread the four scale ops so the PSUM banks release
                        # in parallel (each frees its bank for the next m).
                        # NB GPSIMD/Pool cannot read PSUM on hardware.
                        if tq in (1, 2):
                            nc.scalar.activation(
                                osl, st["ps_o"][tq][:], ACT_ID, scale=rc
                            )
                        else:
                            nc.vector.tensor_scalar_mul(osl, st["ps_o"][tq][:], rc)
                        nc.vector.tensor_add(osl, osl, bv_sb[:])
                    nc.sync.dma_start(
                        out=out[m * 4 * P : (m + 1) * 4 * P, :].rearrange(
                            "(t p) h -> p t h", p=P
                        ),
                        in_=o_sb[:],
                    )

                def open_m(m):
                    state[m] = {
                        "ps_o": [
                            psO.tile([P, D], F32, tag=f"o{t}", name=f"ps_o{t}_{m}")
                            for t in range(4)
                        ],
                        "ps_den": psDen.tile([P, D], F32, tag="dn", name=f"ps_den{m}"),
                    }

                # pipeline: scores(m, jp) run LAG steps ahead of AV(m, jp);
                # the m-tail (which waits on the reciprocal chain) is delayed
                # one further step so the PE never idles on den -> 1/den.
                LAG = 4
                seq = [(m_, jp_) for m_ in range(MC) for jp_ in range(8)]
                open_m(0)
                for k in range(LAG):
                    emit_scores(*seq[k])
                pending_tail = None
                for i, (m, jp) in enumerate(seq):
                    if i + LAG < len(seq):
                        sm, sj = seq[i + LAG]
                        if sj == 0:
                            open_m(sm)
                        emit_scores(sm, sj)
                    if pending_tail is not None:
                        emit_m_tail(pending_tail)
                        pending_tail = None
                    emit_av(m)
                    if jp == 7:
                        pending_tail = m
                emit_m_tail(MC - 1)

        lp.__exit__(None, None, None)

    nc.finalize()
    return nc


def _q8(a):
    return np.ascontiguousarray(a).astype(FP8NP)


def _tile4(a, cols):
    """[512, cols] -> [2, 128, 2, cols] matching the SBUF ff-pair layout."""
    return np.ascontiguousarray(
        a.reshape(2, 2, P, cols).transpose(0, 2, 1, 3)
    )


def _prep_weights(Wq, bq, Wk, bk, Wv, bv):
    Wq = np.asarray(Wq, dtype=np.float64)
    Wk = np.asarray(Wk, dtype=np.float64)
    A = (Wq @ Wk.T).astype(np.float32)
    w3 = (Wk @ np.asarray(bq, dtype=np.float64)).astype(np.float32)
    A8 = _q8(WS * A)
    dA8 = _q8(WS * A - A8.astype(np.float32))
    Wv32 = np.asarray(Wv, dtype=np.float32)
    Wv8 = _q8(WS * Wv32)
    dWv8 = _q8(WS * Wv32 - Wv8.astype(np.float32))
    w38 = _q8(CS * w3)
    return {
        "A8": _tile4(A8, D),
        "dA8": _tile4(dA8, D),
        "Wv8": _tile4(Wv8, D),
        "dWv8": _tile4(dWv8, D),
        "w38": _tile4(w38, 1),
        "bv16": np.ascontiguousarray(
            np.asarray(bv, dtype=np.float16).reshape(1, D)
        ),
    }


def kernel(x, Wq, bq, Wk, bk, Wv, bv):
    x = np.asarray(x, dtype=np.float32)
    wargs = _prep_weights(Wq, bq, Wk, bk, Wv, bv)

    if "nc" not in _CACHE:
        _CACHE["nc"] = _build_nc()
    nc = _CACHE["nc"]

    in_maps = []
    for b in range(B):
        xT = np.ascontiguousarray(x[b].T)
        x8 = xT.astype(FP8NP)
        dx8 = (xT - x8.astype(np.float32)).astype(FP8NP)
        in_maps.append(
            {"x8": _tile4(x8, S), "dx8": _tile4(dx8, S), **wargs}
        )

    try:
        res = run_bass_kernel_spmd(nc, in_maps, list(range(B)))
    except Exception:
        # transient device wedge (e.g. NRT_EXEC_UNIT_UNRECOVERABLE) - retry
        import time as _time

        _time.sleep(5)
        res = run_bass_kernel_spmd(nc, in_maps, list(range(B)))
    return np.stack(
        [np.asarray(res.results[b]["out"]) for b in range(B)]
    ).astype(np.float32)


if __name__ == "__main__":
    rng = np.random.default_rng(0)
    inputs = {
        "x": rng.standard_normal((B, S, D), dtype=np.float32),
        "Wq": rng.standard_normal((D, D), dtype=np.float32) / np.sqrt(D),
        "bq": rng.standard_normal(D).astype(np.float32) * 0.01,
        "Wk": rng.standard_normal((D, D), dtype=np.float32) / np.sqrt(D),
        "bk": rng.standard_normal(D).astype(np.float32) * 0.01,
        "Wv": rng.standard_normal((D, D), dtype=np.float32) / np.sqrt(D),
        "bv": rng.standard_normal(D).astype(np.float32) * 0.01,
    }
    got = kernel(**inputs)
    print("kernel output", got.shape, got.dtype)


# revision 79
# speedup vs baseline: 1.4201x; 1.0020x over previous
"""Trainium2 Bass kernel for single-head attention (fp8 DoubleRow edition).

Problem: x[8, 2048, 512], Wq/Wk/Wv[512, 512], bq/bk/bv[512] ->
out[8, 2048, 512] where out = softmax((xWq+bq)(xWk+bk)^T / sqrt(512)) (xWv+bv).

Sharding: data-parallel over batch; each of the 8 NeuronCores does one batch
element.

Algebraic restructure (host folds weights only):
  scores_ij = (x_i Wq + bq)(x_j Wk + bk)^T
            = x_i A x_j^T + (x_i Wq bk) + x_j (Wk bq) + bq.bk     A = Wq Wk^T
  The 2nd and 4th terms are constant per query row i -> softmax-invariant ->
  dropped. The device computes t = x @ A (ONE projection instead of Q and K),
  a per-key logit bias c = x @ (Wk bq) folded into the exp's bias operand,
  and v = x @ Wv. The output bias bv is softmax-affine (sum of weights = 1)
  and is added as an fp16 broadcast row at output evacuation.

All big matmuls run as fp8e4m3 DoubleRow (perf_mode=DoubleRow: K=256 per
instruction at 0.5 cycles per output column = 4x the f32r MAC rate). fp8
alone is far too coarse (max-rel ~5e-2 vs the 2e-2 gate), so every operand
is a value+residual pair, residual8 = fp8(value - value8) (the PE's fp8
subnormal arithmetic is exact, so residuals need no rescaling). Each
pair x pair product drops the (residual x residual) term -> 3 passes
instead of 4, i.e. 0.75 cycles per K128-row vs 1.0 for f32r/bf16, at ~0.1%
effective precision:
  t   = x8@A8 + dx8@A8 + x8@dA8        (A, Wv prescaled by 32 so their
  v   = x8@Wv8 + dx8@Wv8 + x8@dWv8      residuals stay in fp8 range;
  sT  = x8.t8 + x8.dt8 + dx8.t8         evacuation rescales by 1/32)
  num = (e8+de8)^T v8 + e8^T dv8
  den = sum_j (e8+de8)  via fp8-ones DoubleRow matmuls into a [1,512] row
e8 = fp8(e16), de8 = fp8(e16-e8), e16 = exp(SCALE*sT + c - SHIFT) in fp16.
SHIFT keeps exp below 240 (fp8e4m3 max finite) and cancels in num/den.

Engine budget: exp on ACT (no fast modes); the fp16 intermediates make the
fp8 copies / residual subtracts eligible for the DVE 2x modes, with Pool
taking a share of the phase-1 copies. Tiles are evacuated in PAIRS (one
[P,2,D] two-bank PSUM tile -> one ACT evac + one copy + one subtract).
The phase-2 emission order runs scores LAG=5 tile-pairs ahead of
attention*V and delays each m-chunk's den->1/den->evacuate tail by one
step, so the PE never waits on those chains (PE stalls would also reset
its pstate ramp to 1.2GHz). Host-side input prep: x[b]^T is split into the
fp8 pair (x8, dx8) and laid out exactly as the SBUF tiles so every DMA
line is one 4KB contiguous run per partition.

Hardware-verified: rel err 3.24e-3 vs the fp32 reference; walrus/ISA
constraints found on the way: GPSIMD cannot touch PSUM, DR matmuls need
>=128-partition stationaries (single-row ldweights fails the ISA check),
f32r K=1 transpose matmuls and fp8/f32r memsets are rejected (use f32 +
tensor_copy), and SBUF->SBUF partition-scatter DMA gathers return garbage
on hardware (use K=1 f32 transpose matmuls instead).
"""

import os
import sys

for _p in ("/opt/trn_rl_repo", "/root/.axon_site/_ro/trn_rl_repo"):
    if os.path.isdir(_p) and _p not in sys.path:
        sys.path.append(_p)

import ml_dtypes
import numpy as np

import concourse.bacc as bacc
import concourse.mybir as mybir
import concourse.tile as tile
from concourse.bass_utils import run_bass_kernel_spmd

B = 8
S = 2048
D = 512
P = 128
NT = S // P  # 16 s-tiles
MC = 4  # query chunks of 512
SCALE = 1.0 / float(np.sqrt(D))
SHIFT = 3.0
WS = 32.0  # A/Wv prescale so the weight residuals stay in fp8 normal range
CS = 256.0  # w3 (tiny per-key bias vector) prescale

F32 = mybir.dt.float32
F32R = mybir.dt.float32r
F16 = mybir.dt.float16
F8 = mybir.dt.float8e4
ACT_ID = mybir.ActivationFunctionType.Identity
ACT_EXP = mybir.ActivationFunctionType.Exp
DR = mybir.MatmulPerfMode.DoubleRow
FP8NP = ml_dtypes.float8_e4m3

_CACHE = {}


def _build_nc():
    nc = bacc.Bacc(None)

    # DRAM layouts mirror the SBUF tiles: [ff, p, u, cols] so each partition
    # line is one contiguous run.
    x8d = nc.dram_tensor("x8", [2, P, 2, S], F8, kind="ExternalInput")
    dx8d = nc.dram_tensor("dx8", [2, P, 2, S], F8, kind="ExternalInput")
    A8d = nc.dram_tensor("A8", [2, P, 2, D], F8, kind="ExternalInput")
    dA8d = nc.dram_tensor("dA8", [2, P, 2, D], F8, kind="ExternalInput")
    Wv8d = nc.dram_tensor("Wv8", [2, P, 2, D], F8, kind="ExternalInput")
    dWv8d = nc.dram_tensor("dWv8", [2, P, 2, D], F8, kind="ExternalInput")
    w38d = nc.dram_tensor("w38", [2, P, 2, 1], F8, kind="ExternalInput")
    bvd = nc.dram_tensor("bv16", [1, D], F16, kind="ExternalInput")
    out = nc.dram_tensor("out", [S, D], F16, kind="ExternalOutput")

    with tile.TileContext(nc) as tc:
        lp = nc.allow_low_precision(
            reason="fp8 value+residual pairs carry ~0.1% effective precision"
        )
        lp.__enter__()
        with (
            tc.tile_pool(name="consts", bufs=1) as cp,
            tc.tile_pool(name="tv8", bufs=1) as tv8,
            tc.tile_pool(name="e8p", bufs=5) as e8p,
            tc.tile_pool(name="e16p", bufs=4) as e16p,
            tc.tile_pool(name="tv16", bufs=6) as tv16,
            tc.tile_pool(name="denp", bufs=2) as denp,
            tc.tile_pool(name="outp", bufs=4) as outp,
        ):
            # ---- inputs to SBUF ----
            # DMA order matters: the first projection tiles need x8+A8, then
            # Wv8 (v pass 1), then the residual tensors.
            def mk2(name, cols):
                return [
                    cp.tile([P, 2, cols], F8, tag=f"{name}{ff}", name=f"{name}{ff}")
                    for ff in range(2)
                ]

            x8sb = mk2("x8", S)
            A8sb = mk2("A8", D)
            dx8sb = mk2("dx8", S)
            dA8sb = mk2("dA8", D)
            Wv8sb = mk2("Wv8", D)
            dWv8sb = mk2("dWv8", D)
            w38sb = mk2("w38", 1)

            def load(ts_, dram, ffs=(0, 1)):
                for ff in ffs:
                    nc.sync.dma_start(out=ts_[ff][:], in_=dram[ff])

            load(x8sb, x8d, (0,))
            load(A8sb, A8d)
            load(x8sb, x8d, (1,))
            load(Wv8sb, Wv8d)
            load(dx8sb, dx8d)
            load(dA8sb, dA8d)
            load(dWv8sb, dWv8d)
            load(w38sb, w38d)

            # bv broadcast to all partitions as f16 (added at out-evacuation;
            # f16 everywhere makes the add eligible for the DVE 2x mode)
            bv_sb = cp.tile([P, D], F16, tag="bv", name="bv_sb")
            nc.gpsimd.dma_start(out=bv_sb[:], in_=bvd[0, :].partition_broadcast(P))

            # all-ones [P, 2, 128] stationary for the denominator matmuls
            # (every output partition gets the same column sum; single-row
            # stationaries fail walrus' ldweights ISA check)
            ones32 = cp.tile([P, 2, P], F32, tag="ones32", name="ones32")
            nc.vector.memset(ones32[:], 1.0)
            ones8 = cp.tile([P, 2, P], F8, tag="ones8", name="ones8")
            nc.vector.tensor_copy(ones8[:], ones32[:])
            mov_one = cp.tile([1, 1], F32, tag="mov_one", name="mov_one")
            nc.vector.tensor_copy(mov_one[:], ones32[:1, 0, :1])

            t8sb = [
                tv8.tile([P, 2, S], F8, tag=f"t8_{gg}", name=f"t8_{gg}")
                for gg in range(2)
            ]
            dt8sb = [
                tv8.tile([P, 2, S], F8, tag=f"dt8_{gg}", name=f"dt8_{gg}")
                for gg in range(2)
            ]
            v8sb = [
                tv8.tile([P, 2, D], F8, tag=f"v8_{jp}", name=f"v8_{jp}")
                for jp in range(8)
            ]
            dv8sb = [
                tv8.tile([P, 2, D], F8, tag=f"dv8_{jp}", name=f"dv8_{jp}")
                for jp in range(8)
            ]
            cT_sb = cp.tile([P, NT], F32, tag="cT", name="cT_sb")

            # ---- phase 1: projections (t, v), then the tiny c bias ----
            # tiles are processed in PAIRS: one [P, 2, D] two-bank PSUM tile
            # per pair -> one ACT evac + one fp8 copy + one residual subtract.
            with (
                tc.tile_pool(name="psP", bufs=3, space="PSUM") as psP,
                tc.tile_pool(name="psC", bufs=1, space="PSUM") as psC,
            ):
                evac_n = [0]

                def emit_pair_evac(ps2, sl8, dsl8, name):
                    # ps2 ([P,2,D] f32 PSUM, 32x-scaled) -> fp8 pair via f16
                    t16 = tv16.tile([P, 2, D], F16, tag="t16", name=f"t16_{name}")
                    nc.scalar.activation(t16[:], ps2[:], ACT_ID, scale=1.0 / WS)
                    # balance the fp8-copy between Pool (slow) and DVE (2x mode)
                    if evac_n[0] % 4 < 3:
                        nc.gpsimd.tensor_copy(sl8, t16[:])
                    else:
                        nc.vector.tensor_copy(sl8, t16[:])
                    evac_n[0] += 1
                    nc.vector.tensor_sub(dsl8, t16[:], sl8)

                tsplit = {}

                def emit_t(gg, mc, only=None):
                    # pair of g-tiles (2gg, 2gg+1) for query-chunk mc
                    msl = slice(mc * D, (mc + 1) * D)
                    if only != "rest":
                        ps2 = psP.tile(
                            [P, 2, D], F32, tag="psp", name=f"ps_t{gg}_{mc}"
                        )
                        tsplit[(gg, mc)] = ps2
                    else:
                        ps2 = tsplit.pop((gg, mc))
                    passes = ((A8sb, x8sb), (A8sb, dx8sb), (dA8sb, x8sb))
                    for u in range(2):
                        gt = 2 * gg + u
                        gsl = slice(gt * P, (gt + 1) * P)
                        k = 0
                        for pi, (stat, mov) in enumerate(passes):
                            for ff in range(2):
                                emit_this = (
                                    only is None
                                    or (only == "p1" and pi == 0)
                                    or (only == "rest" and pi > 0)
                                )
                                if emit_this:
                                    nc.tensor.matmul(
                                        ps2[:, u, :],
                                        stat[ff][:, :, gsl],
                                        mov[ff][:, :, msl],
                                        start=(k == 0),
                                        stop=(k == 5),
                                        perf_mode=DR,
                                    )
                                k += 1
                    if only != "p1":
                        emit_pair_evac(
                            ps2,
                            t8sb[gg][:, :, msl],
                            dt8sb[gg][:, :, msl],
                            f"t{gg}_{mc}",
                        )

                def emit_v(jp):
                    # pair of s-tiles (2jp, 2jp+1): fills v8sb[jp]/dv8sb[jp]
                    ps2 = psP.tile([P, 2, D], F32, tag="psp", name=f"ps_v{jp}")
                    for u in range(2):
                        st = 2 * jp + u
                        ssl = slice(st * P, (st + 1) * P)
                        k = 0
                        for stat, mov in (
                            (x8sb, Wv8sb),
                            (dx8sb, Wv8sb),
                            (x8sb, dWv8sb),
                        ):
                            for ff in range(2):
                                nc.tensor.matmul(
                                    ps2[:, u, :],
                                    stat[ff][:, :, ssl],
                                    mov[ff][:],
                                    start=(k == 0),
                                    stop=(k == 5),
                                    perf_mode=DR,
                                )
                                k += 1

                    emit_pair_evac(
                        ps2, v8sb[jp][:], dv8sb[jp][:], f"v{jp}"
                    )

                def emit_c():
                    # per-key logit bias cT[j, tile] = x8 . w38 (256c), landed
                    # directly with keys on partitions via x8-stationary plain
                    # fp8 matmuls (K=128 each, N=1)
                    ps_ct = psC.tile([P, NT], F32, tag="ps_ct", name="ps_ct")
                    for t in range(NT):
                        ssl = slice(t * P, (t + 1) * P)
                        k = 0
                        for ff in range(2):
                            for u in range(2):
                                nc.tensor.matmul(
                                    ps_ct[:, t : t + 1],
                                    x8sb[ff][:, u, ssl],
                                    w38sb[ff][:, u, :],
                                    start=(k == 0),
                                    stop=(k == 3),
                                )
                                k += 1
                    nc.vector.tensor_scalar(
                        cT_sb[:],
                        ps_ct[:],
                        SCALE / CS,
                        -SHIFT,
                        mybir.AluOpType.mult,
                        mybir.AluOpType.add,
                    )

                # interleave so the PE always has a DMA-satisfied group ready.
                # The first two t-groups emit their (x8,A8) pass first so the
                # PE's shallow wait-queue isn't clogged by dx8/dA8-dependent
                # instructions while those DMAs are still in flight.
                emit_t(0, 0, only="p1")
                emit_t(1, 0, only="p1")
                order = [("t", 0, 0), ("v", 0, None),
                         ("t", 1, 0), ("v", 1, None),
                         ("c", None, None), ("v", 2, None), ("v", 3, None)]
                order += [("v", jp, None) for jp in range(4, 8)]
                order += [("t", gg, mc) for mc in range(1, MC) for gg in range(2)]
                for kind_, a, b in order:
                    if kind_ == "t":
                        emit_t(a, b, only="rest" if b == 0 and a in (0, 1) else None)
                    elif kind_ == "c":
                        emit_c()
                    else:
                        emit_v(a)

            # ---- phase 2: scores -> exp pair -> AV + den, software-pipelined ----
            with (
                tc.tile_pool(name="psS", bufs=2, space="PSUM") as psS,
                tc.tile_pool(name="psO", bufs=1, space="PSUM") as psO,
                tc.tile_pool(name="psDen", bufs=1, space="PSUM") as psDen,
                tc.tile_pool(name="psR", bufs=1, space="PSUM") as psR,
            ):
                state = {}  # per-m live objects

                def emit_scores(m, jp):
                    st = state[m]
                    e8t = e8p.tile([P, 2, D], F8, tag="e8", name=f"e8_{m}_{jp}")
                    de8t = e8p.tile([P, 2, D], F8, tag="de8", name=f"de8_{m}_{jp}")
                    e16 = e16p.tile([P, 2, D], F16, tag="e16", name=f"e16_{m}_{jp}")
                    for u in range(2):
                        j16 = jp * 2 + u
                        jsl = slice(j16 * P, (j16 + 1) * P)
                        msl = slice(m * D, (m + 1) * D)
                        ps_s = psS.tile([P, D], F32, tag="s", name=f"ps_s{m}_{j16}")
                        k = 0
                        for stat, mov in (
                            (x8sb, t8sb),
                            (x8sb, dt8sb),
                            (dx8sb, t8sb),
                        ):
                            for ff in range(2):
                                nc.tensor.matmul(
                                    ps_s[:],
                                    stat[ff][:, :, jsl],
                                    mov[ff][:, :, msl],
                                    start=(k == 0),
                                    stop=(k == 5),
                                    perf_mode=DR,
                                )
                                k += 1
                        nc.scalar.activation(
                            e16[:, u, :],
                            ps_s[:],
                            ACT_EXP,
                            scale=SCALE,
                            bias=cT_sb[:, j16 : j16 + 1],
                        )
                    nc.vector.tensor_copy(e8t[:], e16[:])
                    nc.vector.tensor_sub(de8t[:], e16[:], e8t[:])
                    # denominator matmuls ride with the scores (LAG ahead of
                    # AV) so den -> 1/den completes long before the m-tail
                    nc.tensor.matmul(
                        st["ps_den"][:],
                        ones8[:],
                        e8t[:],
                        start=(jp == 0),
                        stop=False,
                        perf_mode=DR,
                    )
                    nc.tensor.matmul(
                        st["ps_den"][:],
                        ones8[:],
                        de8t[:],
                        start=False,
                        stop=(jp == 7),
                        perf_mode=DR,
                    )
                    if jp == 7:
                        rec_row = denp.tile([1, D], F32, tag="rr", name=f"rr{m}")
                        nc.vector.reciprocal(rec_row[:], st["ps_den"][0:1, :])
                        ps_r = psR.tile([P, 4], F32, tag="pr", name=f"ps_r{m}")
                        for tq in range(4):
                            nc.tensor.matmul(
                                ps_r[:, tq : tq + 1],
                                rec_row[:, tq * P : (tq + 1) * P],
                                mov_one[:1, :],
                                start=True,
                                stop=True,
                            )
                        rec_sb = denp.tile([P, 4], F32, tag="rs", name=f"rs{m}")
                        nc.vector.tensor_copy(rec_sb[:], ps_r[:])
                        st["rec_sb"] = rec_sb
                    st.setdefault("pend", []).append((jp, e8t, de8t))

                def emit_av(m):
                    st = state[m]
                    jp, e8t, de8t = st["pend"].pop(0)
                    for tq in range(4):
                        sl = slice(tq * P, (tq + 1) * P)
                        for pi, (stat, mov) in enumerate((
                            (e8t, v8sb[jp]),
                            (de8t, v8sb[jp]),
                            (e8t, dv8sb[jp]),
                        )):
                            nc.tensor.matmul(
                                st["ps_o"][tq][:],
                                stat[:, :, sl],
                                mov[:],
                                start=(jp == 0 and pi == 0),
                                stop=(jp == 7 and pi == 2),
                                perf_mode=DR,
                            )

                def emit_m_tail(m):
                    # transpose 1/den into per-partition columns with a tiny
                    # SBUF->SBUF DMA gather, then scale + DMA out
                    st = state.pop(m)
                    rec_sb = st["rec_sb"]
                    o_sb = outp.tile([P, 4, D], F16, tag="osb", name=f"o{m}")
                    for tq in range(4):
                        osl = o_sb[:, tq, :]
                        rc = rec_sb[:, tq : tq + 1]
                        # spread the four scale ops so the PSUM banks release
                        # in parallel (each frees its bank for the next m).
                        # NB GPSIMD/Pool cannot read PSUM on hardware.
                        if tq in (1, 2):
                            nc.scalar.activation(
                                osl, st["ps_o"][tq][:], ACT_ID, scale=rc
                            )
                        else:
                            nc.vector.tensor_scalar_mul(osl, st["ps_o"][tq][:], rc)
                        nc.vector.tensor_add(osl, osl, bv_sb[:])
                    nc.sync.dma_start(
                        out=out[m * 4 * P : (m + 1) * 4 * P, :].rearrange(
                            "(t p) h -> p t h", p=P
                        ),
                        in_=o_sb[:],
                    )

                def open_m(m):
                    state[m] = {
                        "ps_o": [
                            psO.tile([P, D], F32, tag=f"o{t}", name=f"ps_o{t}_{m}")
                            for t in range(4)
                        ],
                        "ps_den": psDen.tile([P, D], F32, tag="dn", name=f"ps_den{m}"),
                    }

                # pipeline: scores(m, jp) run LAG steps ahead of AV(m, jp);
                # the m-tail (which waits on the reciprocal chain) is delayed
                # one further step so the PE never idles on den -> 1/den.
                LAG = 4
                seq = [(m_, jp_) for m_ in range(MC) for jp_ in range(8)]
                open_m(0)
                for k in range(LAG):
                    emit_scores(*seq[k])
                pending_tail = None
                for i, (m, jp) in enumerate(seq):
                    if i + LAG < len(seq):
                        sm, sj = seq[i + LAG]
                        if sj == 0:
                            open_m(sm)
                        emit_scores(sm, sj)
                    if pending_tail is not None:
                        emit_m_tail(pending_tail)
                        pending_tail = None
                    emit_av(m)
                    if jp == 7:
                        pending_tail = m
                emit_m_tail(MC - 1)

        lp.__exit__(None, None, None)

    nc.finalize()
    return nc


def _q8(a):
    return np.ascontiguousarray(a).astype(FP8NP)


def _tile4(a, cols):
    """[512, cols] -> [2, 128, 2, cols] matching the SBUF ff-pair layout."""
    return np.ascontiguousarray(
        a.reshape(2, 2, P, cols).transpose(0, 2, 1, 3)
    )


def _prep_weights(Wq, bq, Wk, bk, Wv, bv):
    Wq = np.asarray(Wq, dtype=np.float64)
    Wk = np.asarray(Wk, dtype=np.float64)
    A = (Wq @ Wk.T).astype(np.float32)
    w3 = (Wk @ np.asarray(bq, dtype=np.float64)).astype(np.float32)
    A8 = _q8(WS * A)
    dA8 = _q8(WS * A - A8.astype(np.float32))
    Wv32 = np.asarray(Wv, dtype=np.float32)
    Wv8 = _q8(WS * Wv32)
    dWv8 = _q8(WS * Wv32 - Wv8.astype(np.float32))
    w38 = _q8(CS * w3)
    return {
        "A8": _tile4(A8, D),
        "dA8": _tile4(dA8, D),
        "Wv8": _tile4(Wv8, D),
        "dWv8": _tile4(dWv8, D),
        "w38": _tile4(w38, 1),
        "bv16": np.ascontiguousarray(
            np.asarray(bv, dtype=np.float16).reshape(1, D)
        ),
    }


def kernel(x, Wq, bq, Wk, bk, Wv, bv):
    x = np.asarray(x, dtype=np.float32)
    wargs = _prep_weights(Wq, bq, Wk, bk, Wv, bv)

    if "nc" not in _CACHE:
        _CACHE["nc"] = _build_nc()
    nc = _CACHE["nc"]

    in_maps = []
    for b in range(B):
        xT = np.ascontiguousarray(x[b].T)
        x8 = xT.astype(FP8NP)
        dx8 = (xT - x8.astype(np.float32)).astype(FP8NP)
        in_maps.append(
            {"x8": _tile4(x8, S), "dx8": _tile4(dx8, S), **wargs}
        )

    try:
        res = run_bass_kernel_spmd(nc, in_maps, list(range(B)))
    except Exception:
        # transient device wedge (e.g. NRT_EXEC_UNIT_UNRECOVERABLE) - retry
        import time as _time

        _time.sleep(5)
        res = run_bass_kernel_spmd(nc, in_maps, list(range(B)))
    return np.stack(
        [np.asarray(res.results[b]["out"]) for b in range(B)]
    ).astype(np.float32)


if __name__ == "__main__":
    rng = np.random.default_rng(0)
    inputs = {
        "x": rng.standard_normal((B, S, D), dtype=np.float32),
        "Wq": rng.standard_normal((D, D), dtype=np.float32) / np.sqrt(D),
        "bq": rng.standard_normal(D).astype(np.float32) * 0.01,
        "Wk": rng.standard_normal((D, D), dtype=np.float32) / np.sqrt(D),
        "bk": rng.standard_normal(D).astype(np.float32) * 0.01,
        "Wv": rng.standard_normal((D, D), dtype=np.float32) / np.sqrt(D),
        "bv": rng.standard_normal(D).astype(np.float32) * 0.01,
    }
    got = kernel(**inputs)
    print("kernel output", got.shape, got.dtype)
